# revision 1
# baseline (speedup 1.0000x reference)
"""Distributed Trainium2 (Bass/Tile) kernel for a 16-head attention block.

Reference semantics (B=2, S=2048, DIM=1024, H=16, DH=64):
    qkv = x @ w_qkv.T; q,k = rms_norm(.)*w; q,k = rope(q,k)
    attn = softmax(q k^T / sqrt(DH) + mask); out = (attn v) @ w_out.T

Sharding (8 cores): core i -> batch b=i//4, sequence quarter j=i%4 (512 rows).
Each core projects q/k/v for its own 512 rows (fp32), norm+ropes them, then
the 4 cores of a batch group AllGather K and V in bf16 (V carries an extra
all-ones column so the softmax denominator falls out of the P@V matmul).
Attention runs in a transposed layout (scores^T[t, s]) with bf16 matmuls and
an fp32 PSUM; the additive mask is folded into the scores PSUM via an
identity-weighted matmul; softmax skips the max-subtraction (rms-normed
q/k bound |scores| <= sqrt(DH)). The output projection emits out^T
(1024, 512) per core; the host transposes/concats the shards.
"""

import os
import sys

import numpy as np

sys.path.insert(0, "/opt/trn_rl_repo")

import concourse.bass as bass  # noqa: E402
import concourse.mybir as mybir  # noqa: E402
import concourse.tile as tile  # noqa: E402
from concourse import bacc  # noqa: E402
from concourse.masks import make_identity  # noqa: E402

F32 = mybir.dt.float32
BF16 = mybir.dt.bfloat16
AF = mybir.ActivationFunctionType
ALU = mybir.AluOpType

B, S, DIM, H, DH = 2, 2048, 1024, 16, 64
EPS = 1e-6
NCORES = 8
SL = S // 4          # rows per core
NSB = SL // 128      # 128-row s-blocks per core (4)
NDC = DIM // 128     # dim chunks (8)
NTC = S // 128       # t chunks over full sequence (16)
VA = DH + 1          # v augmented with ones column
KV_K = H * DH * SL               # floats in k section per rank
KV_V = SL * H * VA               # floats in v(+ones) section per rank
KV_N = KV_K + KV_V

_CACHE: dict = {}


def _bcast(ap: bass.AP, n: int, axis_pos: int) -> bass.AP:
    """Insert a 0-stride broadcast dim of size n at free-dim position axis_pos."""
    new = list(ap.ap)
    new.insert(axis_pos, [0, n])
    return bass.AP(tensor=ap.tensor, offset=ap.offset, ap=new)


def build(num_cores: int = NCORES, mode: str = "full", causal: bool = False):
    nc = bacc.Bacc(
        "TRN2",
        target_bir_lowering=False,
        debug=False,
        num_devices=num_cores,
    )

    xT_d = nc.dram_tensor("xT", [DIM, SL], F32, kind="ExternalInput")
    wqT_d = nc.dram_tensor("wqT", [DIM, 3 * H * DH], F32, kind="ExternalInput")
    woT_d = nc.dram_tensor("woT", [H * DH, DIM], F32, kind="ExternalInput")
    maskT_d = nc.dram_tensor("maskT", [S, SL], F32, kind="ExternalInput")
    freqs_d = nc.dram_tensor("freqs", [SL, DH // 2], F32, kind="ExternalInput")
    qw_d = nc.dram_tensor("qw", [DH], F32, kind="ExternalInput")
    kw_d = nc.dram_tensor("kw", [DH], F32, kind="ExternalInput")
    outT_d = nc.dram_tensor("outT", [DIM, SL], F32, kind="ExternalOutput")

    groups = [list(range(g * 4, g * 4 + 4)) for g in range(num_cores // 4)] or [[0]]

    with tile.TileContext(nc, num_cores=num_cores) as tc:
        _build_tile(tc, nc, xT_d, wqT_d, woT_d, maskT_d, freqs_d, qw_d, kw_d,
                    outT_d, groups, mode, causal)
    nc.compile()
    return nc


def _build_tile(tc, nc, xT_d, wqT_d, woT_d, maskT_d, freqs_d, qw_d, kw_d,
                outT_d, groups, mode, causal):
    from contextlib import ExitStack

    with ExitStack() as top:
        _build_tile_inner(top, tc, nc, xT_d, wqT_d, woT_d, maskT_d, freqs_d,
                          qw_d, kw_d, outT_d, groups, mode, causal)


def _build_tile_inner(top, tc, nc, xT_d, wqT_d, woT_d, maskT_d, freqs_d,
                      qw_d, kw_d, outT_d, groups, mode, causal):
    from contextlib import ExitStack

    const = top.enter_context(tc.tile_pool(name="const", bufs=1))
    dram = top.enter_context(tc.tile_pool(name="dram", bufs=1, space="DRAM"))

    ident = const.tile([128, 128], F32)
    make_identity(nc, ident[:])
    ident_bf = const.tile([128, 128], BF16)
    nc.vector.tensor_copy(ident_bf[:], ident[:])
    ones128 = const.tile([128, DH], BF16)
    nc.vector.memset(ones128[:], 1.0)
    b_halfpi = const.tile([128, 1], F32)
    nc.vector.memset(b_halfpi[:], float(np.pi / 2))
    b_eps_q = const.tile([128, 1], F32)
    nc.vector.memset(b_eps_q[:], float(DH * EPS))
    b_eps_k = const.tile([128, 1], F32)
    nc.vector.memset(b_eps_k[:], float(EPS))

    # norm weights broadcast to all partitions: [128, DH]
    qw_t = const.tile([128, DH], F32)
    kw_t = const.tile([128, DH], F32)
    nc.sync.dma_start(out=qw_t[:], in_=_bcast(qw_d.ap(), 128, 0))
    nc.sync.dma_start(out=kw_t[:], in_=_bcast(kw_d.ap(), 128, 0))
    # ---- persistent sbuf across stages ----
    persist = top.enter_context(tc.tile_pool(name="persist", bufs=1))
    # q/k in (s, c) layout per s-block, f32 (normed+roped in place)
    qk_sb = [persist.tile([128, 2 * H * DH], F32, name=f"qk{sb}") for sb in range(NSB)]
    # v with ones column, (s, h, d+1), bf16 (shipped through the gather)
    vaug_sb = [persist.tile([128, H, VA], BF16, name=f"va{sb}") for sb in range(NSB)]
    # qT / local-kT head pairs (bf16): partitions = (h%2)*64+d, cols = local s
    qT_sb = [persist.tile([128, SL], BF16, name=f"qT{hp}") for hp in range(H // 2)]
    kT_sb = [persist.tile([128, SL], BF16, name=f"kT{hp}") for hp in range(H // 2)]
    # resident attention-phase tensors
    mT = persist.tile([128, NTC, SL], BF16, name="mT")
    attn_pairs = [persist.tile([128, SL], BF16, name=f"ap{hp}")
                  for hp in range(H // 2)]

    # DRAM bounce buffers for the gather (bf16)
    kv_in = dram.tile([KV_N], BF16)
    k_out = dram.tile([4, KV_K], BF16)
    v_out = dram.tile([4, KV_V], BF16)
    kv_in_k = kv_in[0:KV_K].rearrange("(hp p s) -> hp p s", p=128, s=SL)
    kv_in_v = kv_in[KV_K:KV_N].rearrange("(t h d) -> t h d", h=H, d=VA)

    # ============ stage 1+2+3 fused: projection, norm+rope, transposes ======
    with ExitStack() as st1:
        p1 = st1.enter_context(tc.tile_pool(name="p1", bufs=2))
        p2 = st1.enter_context(tc.tile_pool(name="p2", bufs=2))
        ps1 = st1.enter_context(tc.tile_pool(name="ps1", bufs=3, space="PSUM"))
        ps3 = st1.enter_context(tc.tile_pool(name="ps3", bufs=4, space="PSUM"))

        TWO_PI = float(2 * np.pi)

        def reduced_sin(out_t, src_ap, phase: float):
            # out = sin(src + phase): reduce angle into (-pi, pi] via
            # r = x - int(x / 2pi) * 2pi, plus an is_gt fixup (covers both
            # truncating and rounding float->int conversions).
            y = p2.tile([128, DH // 2], F32, tag="ry")
            nc.vector.tensor_scalar(y[:], src_ap, phase, float(1.0 / TWO_PI),
                                    op0=ALU.add, op1=ALU.mult)
            yi = p2.tile([128, DH // 2], mybir.dt.int32, tag="ryi")
            nc.vector.tensor_copy(yi[:], y[:])
            yf = p2.tile([128, DH // 2], F32, tag="ryf")
            nc.vector.tensor_copy(yf[:], yi[:])
            r = p2.tile([128, DH // 2], F32, tag="rr")
            nc.vector.tensor_scalar(yf[:], yf[:], TWO_PI, None, op0=ALU.mult)
            if phase:
                nc.vector.tensor_scalar(r[:], src_ap, phase, None, op0=ALU.add)
                nc.vector.tensor_tensor(r[:], r[:], yf[:], ALU.subtract)
            else:
                nc.vector.tensor_tensor(r[:], src_ap, yf[:], ALU.subtract)
            m = p2.tile([128, DH // 2], F32, tag="rm")
            nc.vector.tensor_scalar(m[:], r[:], float(np.pi), TWO_PI,
                                    op0=ALU.is_gt, op1=ALU.mult)
            nc.vector.tensor_tensor(r[:], r[:], m[:], ALU.subtract)
            nc.vector.tensor_scalar(r[:], r[:], float(-np.pi), float(np.pi),
                                    op0=ALU.max, op1=ALU.min)
            nc.scalar.activation(out_t[:], r[:], AF.Sin)

        # per-sb cos/sin tiles (persist through the cc loop)
        ct_sb, st_sb, ctb_sb, stb_sb = [], [], [], []
        qkb_sb = [p2.tile([128, 2 * H * DH], BF16, name=f"qkb{sb}", bufs=NSB,
                          tag="qkb") for sb in range(NSB)]
        for sb in range(NSB):
            f_t = p2.tile([128, DH // 2], F32, tag="f")
            nc.sync.dma_start(out=f_t[:], in_=freqs_d[sb * 128:(sb + 1) * 128, :])
            ct = p2.tile([128, DH // 2], F32, name=f"ct{sb}", bufs=NSB, tag="ct")
            st_ = p2.tile([128, DH // 2], F32, name=f"st{sb}", bufs=NSB, tag="st")
            reduced_sin(st_, f_t[:], 0.0)
            reduced_sin(ct, f_t[:], float(np.pi / 2))
            ctb_t = p2.tile([128, DH // 2], BF16, name=f"ctb{sb}", bufs=NSB,
                            tag="ctb")
            stb_t = p2.tile([128, DH // 2], BF16, name=f"stb{sb}", bufs=NSB,
                            tag="stb")
            nc.vector.tensor_copy(ctb_t[:], ct[:])
            nc.vector.tensor_copy(stb_t[:], st_[:])
            ct_sb.append(ct)
            st_sb.append(st_)
            ctb_sb.append(ctb_t)
            stb_sb.append(stb_t)

        # resident multiplicative mask chi = exp(mask) (bf16): downstream,
        # exp(s + m) = exp(s) * chi is applied as a VectorE multiply
        for tcn in range(NTC):
            msk_st = p1.tile([128, SL], F32, tag="mst")
            nc.sync.dma_start(out=msk_st[:],
                              in_=maskT_d[tcn * 128:(tcn + 1) * 128, :])
            nc.scalar.activation(mT[:, tcn, :], msk_st[:], AF.Exp)

        def norm_rope(sb, qk):
            view = qk_sb[sb][:, qk * H * DH:(qk + 1) * H * DH].rearrange(
                "p (h d) -> p h d", h=H)
            sq = p2.tile([128, H, DH], F32, tag="sq")
            nc.scalar.activation(sq[:], view, AF.Square)
            ss = p2.tile([128, H], F32, tag="ss")
            nc.vector.tensor_reduce(ss[:], sq[:], axis=mybir.AxisListType.X,
                                    op=ALU.add)
            rstd = p2.tile([128, H], F32, tag="rstd")
            if qk == 0:
                # fold the 1/sqrt(DH) attention scale into q's rstd
                nc.scalar.activation(rstd[:], ss[:], AF.Sqrt, bias=b_eps_q[:])
            else:
                nc.scalar.activation(rstd[:], ss[:], AF.Sqrt, bias=b_eps_k[:],
                                     scale=float(1.0 / DH))
            nc.vector.reciprocal(rstd[:], rstd[:])
            nc.vector.tensor_tensor(view, view, _bcast(rstd[:], DH, 2), ALU.mult)
            w_t = qw_t if qk == 0 else kw_t
            # the norm-weight multiply also casts to bf16; rope then runs in
            # bf16 at the DVE's 2x mode
            bview = qkb_sb[sb][:, qk * H * DH:(qk + 1) * H * DH].rearrange(
                "p (h d) -> p h d", h=H)
            nc.vector.tensor_tensor(bview, view, _bcast(w_t[:], H, 1), ALU.mult)
            x1 = bview[:, :, 0:DH // 2]
            x2 = bview[:, :, DH // 2:DH]
            ctb = _bcast(ctb_sb[sb][:], H, 1)
            stb = _bcast(stb_sb[sb][:], H, 1)
            a = p2.tile([128, H, DH // 2], BF16, tag="ra")
            b_ = p2.tile([128, H, DH // 2], BF16, tag="rb")
            c_ = p2.tile([128, H, DH // 2], BF16, tag="rc")
            d_ = p2.tile([128, H, DH // 2], BF16, tag="rd")
            nc.vector.tensor_tensor(a[:], x1, ctb, ALU.mult)
            nc.vector.tensor_tensor(b_[:], x2, stb, ALU.mult)
            nc.vector.tensor_tensor(c_[:], x2, ctb, ALU.mult)
            nc.vector.tensor_tensor(d_[:], x1, stb, ALU.mult)
            nc.vector.tensor_tensor(x1, a[:], b_[:], ALU.subtract)
            nc.vector.tensor_tensor(x2, c_[:], d_[:], ALU.add)

        def transpose_pairs(sb, qk):
            # [s=128, (2h,d)=128] -> [(2h,d), s], evicted as bf16
            dst = qT_sb if qk == 0 else kT_sb
            for hp in range(H // 2):
                pt = ps3.tile([128, 128], BF16, tag="pt")
                nc.tensor.transpose(
                    pt[:],
                    qkb_sb[sb][:, qk * H * DH + hp * 128:
                               qk * H * DH + (hp + 1) * 128],
                    ident_bf[:])
                nc.scalar.copy(dst[hp][:, sb * 128:(sb + 1) * 128], pt[:])

        # projection: x^T and w chunks cast to bf16 on the fly
        xT_sb = [p1.tile([128, SL], BF16, name=f"xT{dc}", tag="xT", bufs=NDC)
                 for dc in range(NDC)]
        for dc in range(NDC):
            xst = p1.tile([128, SL], F32, tag="xst")
            nc.sync.dma_start(out=xst[:], in_=xT_d[dc * 128:(dc + 1) * 128, :])
            nc.gpsimd.tensor_copy(xT_sb[dc][:], xst[:])

        NCC = (3 * H * DH) // 512  # 6 chunks of 512 output channels
        for cc in range(NCC):
            wq_st = p1.tile([128, NDC, 512], F32, tag="wqst", bufs=2)
            nc.sync.dma_start(
                out=wq_st[:],
                in_=bass.AP(tensor=wqT_d, offset=cc * 512,
                            ap=[[3 * H * DH, 128], [128 * 3 * H * DH, NDC],
                                [1, 512]]),
            )
            wq_cc = p1.tile([128, NDC, 512], BF16, tag="wq")
            if cc % 2 == 0:
                nc.vector.tensor_copy(wq_cc[:], wq_st[:])
            else:
                nc.scalar.copy(wq_cc[:], wq_st[:])
            for sb in range(NSB):
                ps = ps1.tile([128, 512], F32, tag="ps")
                for dc in range(NDC):
                    nc.tensor.matmul(
                        ps[:],
                        xT_sb[dc][:, sb * 128:(sb + 1) * 128],
                        wq_cc[:, dc, :],
                        start=(dc == 0),
                        stop=(dc == NDC - 1),
                    )
                if cc < 4:  # q,k channels
                    nc.scalar.copy(qk_sb[sb][:, cc * 512:(cc + 1) * 512], ps[:])
                else:  # v channels -> (h, d) slots of vaug (cast to bf16)
                    h0 = (cc - 4) * 8
                    nc.scalar.copy(
                        vaug_sb[sb][:, h0:h0 + 8, 0:DH],
                        ps[:].rearrange("p (h d) -> p h d", h=8),
                    )
                # as soon as the q (or k) half of this s-block is complete,
                # run its norm+rope and transposes — overlaps later projection
                if cc == 1:
                    norm_rope(sb, 0)
                    transpose_pairs(sb, 0)
                elif cc == 3:
                    norm_rope(sb, 1)
                    transpose_pairs(sb, 1)

        for hp in range(H // 2):
            nc.sync.dma_start(out=kv_in_k[hp], in_=kT_sb[hp][:])
        if mode == "full":
            nc.gpsimd.collective_compute(
                "AllGather", ALU.bypass, replica_groups=groups,
                ins=[kv_in[0:KV_K].opt()],
                outs=[k_out[:].opt()])
        for sb in range(NSB):
            nc.vector.memset(vaug_sb[sb][:, :, DH:VA], 1.0)
            nc.sync.dma_start(out=kv_in_v[sb * 128:(sb + 1) * 128],
                              in_=vaug_sb[sb][:])

    # late pool reuses stage-1's sbuf space (stack allocator, LIFO)
    late = top.enter_context(tc.tile_pool(name="late", bufs=1))
    v_full = late.tile([128, NTC, H, VA], BF16, name="v_full")
    woT_sb = [late.tile([128, DIM], BF16, name=f"wo{hp}")
              for hp in range(H // 2)]
    # out-proj weight loads: independent, overlap the gather + attention
    for hp in range(H // 2):
        wst = late.tile([128, DIM], F32, name=f"wst{hp}", tag="wst", bufs=2)
        nc.sync.dma_start(out=wst[:], in_=woT_d[hp * 128:(hp + 1) * 128, :])
        nc.gpsimd.tensor_copy(woT_sb[hp][:], wst[:])

    if mode == "full":
        nc.gpsimd.collective_compute(
            "AllGather",
            ALU.bypass,
            replica_groups=groups,
            ins=[kv_in[KV_K:KV_N].opt()],
            outs=[v_out[:].opt()],
        )
    else:
        # profiling variant: stand in for the AllGather with 4 local
        # DRAM->DRAM copies (same downstream structure, wrong data for
        # ranks != self — used only for engine-occupancy profiling)
        for r in range(4):
            nc.sync.dma_start(out=k_out[r], in_=kv_in[0:KV_K])
            nc.sync.dma_start(out=v_out[r], in_=kv_in[KV_K:KV_N])

    # ============ stage 4: attention (bf16 matmuls, fp32 psum) ==============
    # causal mode (strided row sharding, rows j::4 per core): for t-chunk tc
    # only local-s columns >= 32*tc can be unmasked — identical on every core
    # — so scores/exp/PV are restricted to the live column range. Entries
    # inside the live range still get the data-driven chi multiply.
    with ExitStack() as st4:
        p4 = st4.enter_context(tc.tile_pool(name="p4", bufs=2))
        ps4 = st4.enter_context(tc.tile_pool(name="ps4", bufs=2, space="PSUM"))
        pso = st4.enter_context(tc.tile_pool(name="pso", bufs=2, space="PSUM"))
        psb = st4.enter_context(tc.tile_pool(name="psb", bufs=1, space="PSUM"))

        for tcn in range(NTC):
            if causal:
                if tcn > 0:
                    continue
                # chunk tc rows = (r, i' in [32tc, 32tc+32)): one DMA per
                # rank covering all chunks at partition offset 32r
                for r in range(4):
                    nc.sync.dma_start(
                        out=v_full[32 * r:32 * (r + 1), :, :, :],
                        in_=v_out[r].rearrange(
                            "(tcn t h d) -> t tcn h d", tcn=NTC, t=32, h=H),
                    )
            else:
                r, lo = tcn // NSB, (tcn % NSB) * 128
                nc.sync.dma_start(
                    out=v_full[:, tcn, :, :],
                    in_=v_out[r, lo * H * VA:(lo + 128) * H * VA].rearrange(
                        "(t h d) -> t h d", h=H, d=VA),
                )

        def off_of(tcn):
            return 32 * tcn if causal else 0

        NP = NTC // 2
        for hp in range(H // 2):
            # gathered k for a head pair: partitions = ((h%2), d)
            if causal:
                # [d, tc, slot] with slot = 32*r + i' — each chunk's 128
                # t-slots contiguous so the matmul weights AP stays 1-D
                kT_hp = p4.tile([128, NTC, 128], BF16, tag="kTh")
                for r in range(4):
                    nc.sync.dma_start(
                        out=kT_hp[:, :, 32 * r:32 * (r + 1)],
                        in_=k_out[r, hp * 128 * SL:(hp + 1) * 128 * SL]
                        .rearrange("(d tcn i) -> d tcn i", tcn=NTC, i=32),
                    )
            else:
                kT_hp = p4.tile([128, 4, SL], BF16, tag="kTh")
                for r in range(4):
                    nc.sync.dma_start(
                        out=kT_hp[:, r, :],
                        in_=k_out[r, hp * 128 * SL:(hp + 1) * 128 * SL]
                        .rearrange("(d s) -> d s", s=SL),
                    )

            def kchunk(par, tcn):
                if causal:
                    return kT_hp[par:par + DH, tcn, :]
                r, lo = tcn // NSB, (tcn % NSB) * 128
                return kT_hp[par:par + DH, r, lo:lo + 128]

            for sub in range(2):
                h = 2 * hp + sub
                par = sub * DH
                po = pso.tile([VA, SL], F32, tag="po")
                pes = {}
                # software-pipelined with a 2-pair lag: the PV matmuls for
                # pair tp-2 run while pair tp traverses exp (ACT) and the
                # chi multiply (DVE) — hides the cross-engine chain latency
                for tp in range(NP + 2):
                    if tp < NP:
                        o = off_of(2 * tp)  # even-half (wider) offset
                        ps = ps4.tile([128, 2, SL], F32, tag="pscore")
                        pe = p4.tile([128, 2, SL], BF16, tag="pexp", bufs=4)
                        for half in range(2):
                            tcn = 2 * tp + half
                            # both halves span the even-half (wider) range so
                            # every psum element read by the exp is written;
                            # chi zeroes the odd half's sub-diagonal band
                            nc.tensor.matmul(
                                ps[:, half, o:SL],
                                kchunk(par, tcn),
                                qT_sb[hp][par:par + DH, o:SL],
                                start=True, stop=True)
                        nc.scalar.activation(pe[:, :, o:SL], ps[:, :, o:SL],
                                             AF.Exp)
                        nc.vector.tensor_tensor(pe[:, :, o:SL], pe[:, :, o:SL],
                                                mT[:, 2 * tp:2 * tp + 2, o:SL],
                                                ALU.mult)
                        pes[tp] = pe
                    if tp >= 2:
                        pe_prev = pes.pop(tp - 2)
                        o = off_of(2 * (tp - 2))
                        for half in range(2):
                            tcn = 2 * (tp - 2) + half
                            nc.tensor.matmul(po[:, o:SL],
                                             v_full[:, tcn, h, :],
                                             pe_prev[:, half, o:SL],
                                             start=(tcn == 0),
                                             stop=(tcn == NTC - 1))
                # epilogue: normalize by the ones-column denominator
                acc = p4.tile([VA, SL], F32, tag="acc")
                nc.vector.tensor_copy(acc[:], po[:])
                # full-tile (base-0) custom-DVE op: the uop misbehaves at a
                # non-zero partition base; rows 0:64 of rcp are discarded
                rcp = p4.tile([VA, SL], F32, tag="rcp")
                nc.vector.reciprocal_approx_fast(rcp[:], acc[:])
                rd_bf = p4.tile([VA, SL], BF16, tag="rdbf")
                nc.vector.tensor_copy(rd_bf[DH:VA, :], rcp[DH:VA, :])
                pb = psb.tile([DH, SL], F32, tag="pb")
                nc.tensor.matmul(pb[:], ones128[DH:DH + 1, :], rd_bf[DH:VA, :],
                                 start=True, stop=True)
                an = p4.tile([DH, SL], BF16, tag="an")
                nc.vector.tensor_tensor(an[:], acc[0:DH, :], pb[:], ALU.mult)
                if sub == 0:
                    nc.vector.tensor_copy(attn_pairs[hp][0:DH, :], an[:])
                else:
                    nc.sync.dma_start(out=attn_pairs[hp][DH:128, :], in_=an[:])

    # ============ stage 5: output projection (emits out^T) ==============
    with ExitStack() as st5:
        p5 = st5.enter_context(tc.tile_pool(name="p5", bufs=3))
        ps5 = st5.enter_context(tc.tile_pool(name="ps5", bufs=3, space="PSUM"))
        for oc in range(NDC):
            pf = ps5.tile([128, SL], F32, tag="pf")
            for hp in range(H // 2):
                nc.tensor.matmul(pf[:], woT_sb[hp][:, oc * 128:(oc + 1) * 128],
                                 attn_pairs[hp][:],
                                 start=(hp == 0), stop=(hp == H // 2 - 1))
            of = p5.tile([128, SL], F32, tag="of")
            nc.scalar.copy(of[:], pf[:])
            nc.sync.dma_start(out=outT_d[oc * 128:(oc + 1) * 128, :], in_=of[:])


def _get_nc(causal: bool):
    key = f"nc_causal{causal}"
    if key not in _CACHE:
        _CACHE[key] = build(causal=causal)
    return _CACHE[key]


def mask_is_causal(mask) -> bool:
    """True if every strictly-future entry (t > s) is <= -60 — the condition
    under which the strided-causal kernel's skipped region contributes 0."""
    m = np.asarray(mask, np.float32).reshape(S, S)
    iu = np.triu_indices(S, 1)
    return bool(np.all(m[iu] <= -60.0))


def make_in_maps(x, mask, rope_freqs, w_qkv, w_out, q_norm_w, k_norm_w,
                 causal: bool):
    x = np.asarray(x, np.float32)
    mask = np.asarray(mask, np.float32)
    rope_freqs = np.asarray(rope_freqs, np.float32)
    wqT = np.ascontiguousarray(np.asarray(w_qkv, np.float32).T)
    woT = np.ascontiguousarray(np.asarray(w_out, np.float32).T)
    qw = np.ascontiguousarray(np.asarray(q_norm_w, np.float32))
    kw = np.ascontiguousarray(np.asarray(k_norm_w, np.float32))
    in_maps = []
    for i in range(NCORES):
        b, j = i // 4, i % 4
        rows = slice(j, None, 4) if causal else slice(j * SL, (j + 1) * SL)
        mT = np.ascontiguousarray(mask[0, 0, rows, :].T)  # [t, s_local]
        if causal:
            # permute t into the gathered chunk-slot order: slot (c, r, i')
            # holds global t = 128*c + 4*i' + r
            mT = np.ascontiguousarray(
                mT.reshape(NTC, 32, 4, SL).transpose(0, 2, 1, 3).reshape(S, SL))
        in_maps.append({
            "xT": np.ascontiguousarray(x[b, rows, :].T),
            "wqT": wqT,
            "woT": woT,
            "maskT": mT,
            "freqs": np.ascontiguousarray(rope_freqs[rows, :DH // 2]),
            "qw": qw,
            "kw": kw,
        })
    return in_maps


def assemble(results, causal: bool):
    out = np.empty((B, S, DIM), np.float32)
    for i in range(NCORES):
        b, j = i // 4, i % 4
        rows = slice(j, None, 4) if causal else slice(j * SL, (j + 1) * SL)
        out[b, rows, :] = results[i]["outT"].T
    return out


LAST_EXEC_TIME_NS = None


def _install_ntff_shim():
    """Register the axon NTFF profile hook (missing antenv.axon_hooks shim)."""
    import sys as _sys
    import types

    if "antenv.axon_hooks" in _sys.modules:
        return
    try:
        _sys.path.insert(0, "/root/.axon_site")
        from trn_agent_boot.trn_boot import _ntff_profile_via_ctypes

        hook = _ntff_profile_via_ctypes("/opt/axon/libaxon_pjrt.so")
        mod = types.ModuleType("antenv.axon_hooks")
        mod.get_axon_ntff_profile_hook = lambda: hook
        mod.set_axon_ntff_profile_hook = lambda h: None
        _sys.modules["antenv.axon_hooks"] = mod
    except Exception as e:  # profiling is best-effort
        print(f"ntff shim failed: {e}")


def kernel(x, mask, rope_freqs, w_qkv, w_out, q_norm_w, k_norm_w):
    global LAST_EXEC_TIME_NS
    from concourse.bass_utils import run_bass_kernel_spmd

    causal = mask_is_causal(mask)
    nc = _get_nc(causal)
    in_maps = make_in_maps(x, mask, rope_freqs, w_qkv, w_out, q_norm_w,
                           k_norm_w, causal)
    trace = bool(int(os.environ.get("KERNEL_TRACE", "0")))
    if trace:
        _install_ntff_shim()
    tcores = os.environ.get("KERNEL_TRACE_CORES")
    res = run_bass_kernel_spmd(
        nc, in_maps, core_ids=list(range(NCORES)), trace=trace,
        trace_cores=[int(c) for c in tcores.split(",")] if tcores else None,
    )
    LAST_EXEC_TIME_NS = res.exec_time_ns
    return assemble(res.results, causal)

